# revision 18
# baseline (speedup 1.0000x reference)
"""Trainium2 Bass kernel for nn_AttentionBlock (GroupNorm + qkv conv + head-dim attention + proj + residual).

Sharding: data-parallel over batch B=16 -> 2 batch elements per core on 8 cores.

Structure (per batch element). The attention contracts over PIXELS (scores are
[64,64] per head), so q,k,v are never materialized per-pixel:
  G    = X X^T            bf16 Gram from DMA-transposed x chunks (no PE
                          transposes, no engine transpose copies)
  stats: channel sums ride the Gram as 4 extra ones-columns; channel sum(x^2)
         comes off the Gram diagonal (diag-block * I, row-reduce).  GroupNorm
         mean/rstd via the gmask matmuls.  No bn_stats pass over x.
  Tk   = G Wk'^T + Sx (x) Bk    (f32r, exact in sim)
  S_p  = Wq'^T Tk + Bq (x) hk   per-head-pair scores (f32r)
  E    = softmax(S/8)           rden folded into E (bf16)
  UT   = E'^T Wp^T ; MT = Wv'^T UT  -> M8 = fp8(32*MT), Mlo = fp8(32*MT - M8)
  out  = [M8^T(x8+e8) + Mlo^T x8]/32 + tbias + residual
         3 fp8 DoubleRow chains (2 steps each) instead of 4 bf16 steps.
         x8 = fp8(x), e8 = fp8(x - x8) are host-prepared; residual lands in
         out2 via an early DRAM->DRAM cast copy, and the projection output is
         DMA-accumulated on top (gpsimd SWDGE).
GroupNorm is folded into the weights (Wq' = Wq diag(a), biases via b2 = beta -
mean*a); x is never normalized in memory.
"""
import sys, os
sys.path.insert(0, "/opt/trn_rl_repo")
sys.path.insert(0, "/opt/trn_rl_repo/concourse")
import numpy as np

B, C, H, W = 16, 512, 64, 64
N = H * W            # 4096 spatial
NH = 8               # heads
D = C // NH          # 64 head dim
G = 32               # groups
EPS = 1e-5
NCORES = 8
BPC = B // NCORES    # 2 batches per core

NT = C // 128        # 4 channel tiles
NCHUNK = N // 128    # 32 pixel chunks
NJ = N // 512        # 8 column blocks of 512
SS = 32.0            # fp8 M scale

_cache = {}


def _build():
    import concourse.bass as bass
    import concourse.bacc as bacc
    import concourse.tile as tile
    from concourse import mybir
    from concourse.masks import make_identity

    f32 = mybir.dt.float32
    f32r = mybir.dt.float32r
    bf16 = mybir.dt.bfloat16
    fp8 = mybir.dt.float8e4
    AF = mybir.ActivationFunctionType
    ALU = mybir.AluOpType
    AX = mybir.AxisListType
    DR = mybir.MatmulPerfMode.DoubleRow

    nc = bacc.Bacc()

    x2bf = nc.dram_tensor("x2bf", [BPC, C, N], bf16, kind="ExternalInput")
    x8d = nc.dram_tensor("x8d", [BPC, C, N], fp8, kind="ExternalInput")
    e8d = nc.dram_tensor("e8d", [BPC, C, N], fp8, kind="ExternalInput")
    wqkT = nc.dram_tensor("wqkT", [C, 3 * C], f32r, kind="ExternalInput")  # w_qkv.T  [c, o]
    wpT = nc.dram_tensor("wpT", [C, C], f32, kind="ExternalInput")          # w_proj.T [c, o]
    gamma_pc = nc.dram_tensor("gamma_pc", [128, NT], f32, kind="ExternalInput")
    beta_pc = nc.dram_tensor("beta_pc", [128, NT], f32, kind="ExternalInput")
    bqk_row = nc.dram_tensor("bqk_row", [1, 2 * C], f32, kind="ExternalInput")
    bv_pc = nc.dram_tensor("bv_pc", [128, NT], f32, kind="ExternalInput")
    bp_pc = nc.dram_tensor("bp_pc", [128, NT], f32, kind="ExternalInput")
    ident_d = nc.dram_tensor("ident_d", [128, 128], f32r, kind="ExternalInput")
    gmask_d = nc.dram_tensor("gmask_d", [128, 8], f32r, kind="ExternalInput")
    gmaskT_d = nc.dram_tensor("gmaskT_d", [8, 128], f32r, kind="ExternalInput")
    out2 = nc.dram_tensor("out2", [BPC, C, N], bf16, kind="ExternalOutput")

    GXW = [512, 384, 256, 128]   # true upper-triangle widths per row block

    with tile.TileContext(nc) as tc:
        with tc.tile_pool(name="consts", bufs=1) as consts, \
             tc.tile_pool(name="wpool", bufs=1) as wpool, \
             tc.tile_pool(name="xpool", bufs=1) as xpool, \
             tc.tile_pool(name="gpool", bufs=1) as gpool, \
             tc.tile_pool(name="xtcpool", bufs=1) as xtcpool, \
             tc.tile_pool(name="rows", bufs=1) as rows, \
             tc.tile_pool(name="work", bufs=2) as work, \
             tc.tile_pool(name="stagepool", bufs=2) as stagepool, \
             tc.tile_pool(name="ps", bufs=1, space="PSUM") as ps:

            # ---------------- constants / weights (once per core) ----------------
            ident = consts.tile([128, 128], f32, tag="ident")
            make_identity(nc, ident)
            identr = consts.tile([128, 128], f32r, tag="identr")
            nc.scalar.dma_start(out=identr, in_=ident_d[:, :])
            gmask = consts.tile([128, 8], f32r, tag="gmask")
            nc.scalar.dma_start(out=gmask, in_=gmask_d[:, :])
            gmaskT = consts.tile([8, 128], f32r, tag="gmaskT")
            nc.scalar.dma_start(out=gmaskT, in_=gmaskT_d[:, :])
            onescol = consts.tile([128, 1], bf16, tag="onescol")
            nc.vector.memset(onescol, 1.0)
            epst8 = consts.tile([8, 1], f32, tag="epst8")
            nc.vector.memset(epst8, EPS)
            # residual identity for the fp8 DoubleRow GEMM: [:, 0:2, :] selects
            # (32*I, 0) for even m blocks, [:, 1:3, :] selects (0, 32*I) for odd.
            I32 = consts.tile([128, 3, 128], fp8, tag="I32")
            nc.vector.memset(I32, 0.0)
            with nc.allow_low_precision(reason="fp8 exact powers of two"):
                nc.scalar.activation(out=I32[:, 0, :], in_=ident, func=AF.Copy, scale=SS)
                nc.scalar.activation(out=I32[:, 2, :], in_=ident, func=AF.Copy, scale=SS)

            bpc_t = consts.tile([128, NT], f32, tag="bpc_t")
            nc.scalar.dma_start(out=bpc_t, in_=bp_pc[:, :])
            gam = consts.tile([128, NT], f32, tag="gam")
            nc.gpsimd.dma_start(out=gam, in_=gamma_pc[:, :])
            bet = consts.tile([128, NT], f32, tag="bet")
            nc.gpsimd.dma_start(out=bet, in_=beta_pc[:, :])
            bvc = consts.tile([128, NT], f32, tag="bvc")
            nc.gpsimd.dma_start(out=bvc, in_=bv_pc[:, :])
            bqkr = consts.tile([1, 2 * C], f32, tag="bqkr")
            nc.gpsimd.dma_start(out=bqkr, in_=bqk_row[:, :])

            wtmp = []
            ws = []
            wp = []
            for t in range(NT):
                w_t = wpool.tile([128, 3 * C], f32r, tag=f"wtmp{t}", name=f"wtmp{t}")
                nc.gpsimd.dma_start(out=w_t, in_=wqkT[128 * t:128 * (t + 1), :])
                wtmp.append(w_t)
                ws_t = wpool.tile([128, 3 * C], f32r, tag=f"ws{t}", name=f"ws{t}")
                ws.append(ws_t)
                w_p = wpool.tile([128, C], bf16, tag=f"wp{t}", name=f"wp{t}")
                nc.gpsimd.dma_start(out=w_p, in_=wpT[128 * t:128 * (t + 1), :])
                wp.append(w_p)

            for b in range(BPC):
                # ---------------- DMA: transposed x chunks + fp8 x loads ----------------
                PREF = 8
                xtc_l = []

                def emit_xtc(ni):
                    xtc = xtcpool.tile([128, C], bf16, tag=f"xtc{ni}", name=f"xtc{ni}")
                    eng = nc.sync if ni % 2 == 0 else nc.scalar
                    eng.dma_start(out=xtc,
                                  in_=x2bf[b, :, 128 * ni:128 * (ni + 1)],
                                  transpose=True)
                    xtc_l.append(xtc)

                for ni in range(PREF):
                    emit_xtc(ni)

                x8t = [[None] * 4 for _ in range(2)]
                e8t = [[None] * 4 for _ in range(2)]

                def emit_x8(h, q):
                    x_t = xpool.tile([128, 2, 1024], fp8, tag=f"x8{h}{q}",
                                     name=f"x8{h}{q}", bufs=1)
                    nc.sync.dma_start(
                        out=x_t,
                        in_=x8d[b, 256 * h:256 * (h + 1), 1024 * q:1024 * (q + 1)]
                        .rearrange("(i k) n -> k i n", i=2))
                    x8t[h][q] = x_t
                    e_t = xpool.tile([128, 2, 1024], fp8, tag=f"e8{h}{q}",
                                     name=f"e8{h}{q}", bufs=1)
                    nc.gpsimd.dma_start(
                        out=e_t,
                        in_=e8d[b, 256 * h:256 * (h + 1), 1024 * q:1024 * (q + 1)]
                        .rearrange("(i k) n -> k i n", i=2))
                    e8t[h][q] = e_t

                # ---------------- Gram (bf16) + channel-sum columns ----------------
                # gxA: rows 0:128  cols 0:512   (bank 1)
                # gxB: rows 128:256 cols 128:512 (bank 2)
                # gxCD: rows 256:384 cols 256:512 at [:,0:256];
                #       rows 384:512 cols 384:512 at [:,256:384];
                #       channel sums at [:,384:388]          (bank 3)
                gxA = ps.tile([128, 512], f32, tag="gxA", name="gxA", bufs=1)
                gxB = ps.tile([128, 512], f32, tag="gxB", name="gxB", bufs=1)
                gxCD = ps.tile([128, 512], f32, tag="gxCD", name="gxCD", bufs=1)

                for ni in range(NCHUNK):
                    if ni + PREF < NCHUNK:
                        emit_xtc(ni + PREF)
                    elif ni + PREF < NCHUNK + 8:
                        j = ni + PREF - NCHUNK
                        emit_x8(j >> 2, j & 3)
                    xtc = xtc_l[ni]
                    st = (ni == 0)
                    sp = (ni == NCHUNK - 1)
                    nc.tensor.matmul(gxA, xtc[:, 0:128], xtc[:, 0:512],
                                     start=st, stop=sp, skip_group_check=True)
                    nc.tensor.matmul(gxB[:, 0:384], xtc[:, 128:256], xtc[:, 128:512],
                                     start=st, stop=sp, skip_group_check=True)
                    nc.tensor.matmul(gxCD[:, 0:256], xtc[:, 256:384], xtc[:, 256:512],
                                     start=st, stop=False, skip_group_check=True)
                    nc.tensor.matmul(gxCD[:, 256:384], xtc[:, 384:512], xtc[:, 384:512],
                                     start=False, stop=False, skip_group_check=True)
                    for cb in range(NT):
                        nc.tensor.matmul(gxCD[:, 384 + cb:385 + cb],
                                         xtc[:, 128 * cb:128 * (cb + 1)], onescol,
                                         start=False, stop=sp and (cb == NT - 1),
                                         skip_group_check=True)

                # ---------------- drain G to SBUF (f32r), sums to S8 ----------------
                gx_src = [gxA[:, 0:512], gxB[:, 0:384], gxCD[:, 0:256], gxCD[:, 256:384]]
                gs = []
                for cb in range(NT):
                    g_s = gpool.tile([128, GXW[cb]], f32r, tag=f"gs{cb}", name=f"gs{cb}")
                    if cb % 2 == 0:
                        nc.scalar.activation(out=g_s, in_=gx_src[cb], func=AF.Identity)
                    else:
                        nc.vector.tensor_copy(g_s, gx_src[cb])
                    gs.append(g_s)
                S8 = work.tile([128, 8], f32r, tag="S8", bufs=1)
                with nc.allow_low_precision(reason="sums feed f32r matmuls"):
                    nc.scalar.activation(out=S8[:, 0:4], in_=gxCD[:, 384:388], func=AF.Identity)
                    # diag(G) per row block: mask with identity, row-reduce
                    for cb in range(NT):
                        dsq = work.tile([128, 128], f32r, tag="dsq", name="dsq", bufs=2)
                        nc.vector.tensor_tensor(dsq, gs[cb][:, 0:128], ident, op=ALU.mult)
                        nc.vector.reduce_sum(out=S8[:, 4 + cb:5 + cb], in_=dsq, axis=AX.X)

                # ---------------- group stats via mask matmuls ----------------
                gsum_ps = ps.tile([8, 8], f32, tag="small", name="gsum_ps", bufs=1,
                                  padded_shape=[8, 512])
                nc.tensor.matmul(gsum_ps, gmask, S8, start=True, stop=True,
                                 skip_group_check=True)
                mg8 = work.tile([8, 8], f32r, tag="mg8")
                with nc.allow_low_precision(reason="feeds f32r matmul"):
                    nc.scalar.mul(out=mg8[:, 0:4], in_=gsum_ps[:, 0:4], mul=1.0 / (16.0 * N))
                ex2 = work.tile([8, 4], f32, tag="ex2")
                nc.scalar.mul(out=ex2, in_=gsum_ps[:, 4:8], mul=1.0 / (16.0 * N))
                msq = work.tile([8, 4], f32, tag="msq")
                nc.vector.tensor_tensor(msq, mg8[:, 0:4].bitcast(f32), mg8[:, 0:4].bitcast(f32),
                                        op=ALU.mult)
                var_g = work.tile([8, 4], f32, tag="var_g")
                nc.vector.tensor_tensor(var_g, ex2, msq, op=ALU.subtract)
                sdg = work.tile([8, 4], f32, tag="sdg")
                nc.scalar.activation(out=sdg, in_=var_g, func=AF.Sqrt, bias=epst8)
                with nc.allow_low_precision(reason="feeds f32r matmul"):
                    nc.vector.reciprocal(mg8[:, 4:8], sdg)
                pcmr = ps.tile([128, 8], f32, tag="small", name="pcmr", bufs=1,
                               padded_shape=[128, 512])
                nc.tensor.matmul(pcmr, gmaskT, mg8, start=True, stop=True,
                                 skip_group_check=True)
                acol = work.tile([128, NT], f32, tag="acol")
                nc.vector.tensor_tensor(acol, pcmr[:, 4:8], gam, op=ALU.mult)
                # bsx cols 0:4 = b2 = beta - mean_g*a ; cols 4:8 = b2 + a*mean_c
                bsx = rows.tile([128, 8], f32r, tag="bsx")
                tmpb = work.tile([128, NT], f32, tag="tmpb")
                nc.vector.tensor_tensor(tmpb, pcmr[:, 0:4], acol, op=ALU.mult)
                with nc.allow_low_precision(reason="feeds f32r matmul"):
                    nc.vector.tensor_tensor(bsx[:, 0:4], bet, tmpb, op=ALU.subtract)
                amv = work.tile([128, NT], f32, tag="amv")
                nc.vector.tensor_tensor(amv, acol, S8[:, 0:4].bitcast(f32), op=ALU.mult)
                with nc.allow_low_precision(reason="feeds f32r matmul"):
                    nc.vector.scalar_tensor_tensor(
                        out=bsx[:, 4:8], in0=amv, scalar=1.0 / N,
                        in1=bsx[:, 0:4].bitcast(f32), op0=ALU.mult, op1=ALU.add)
                # channel-sum rows for the rank-1 score terms
                sxrow_l = []
                for t in range(NT):
                    sxtp = ps.tile([1, 128], f32, tag="small", name="sxtp", bufs=1,
                                   padded_shape=[1, 512])
                    nc.tensor.transpose(sxtp, S8[:, t:t + 1].bitcast(f32), ident)
                    sxrow = rows.tile([1, 128], f32r, tag=f"sxrow{t}", name=f"sxrow{t}")
                    with nc.allow_low_precision(reason="feeds f32r matmul"):
                        nc.scalar.mul(out=sxrow, in_=sxtp, mul=1.0)
                    sxrow_l.append(sxrow)

                # ---------------- ws = wtmp * acol (split ACT/DVE) ----------------
                for t in range(NT):
                    if t % 2 == 0:
                        nc.scalar.activation(out=ws[t], in_=wtmp[t], func=AF.Copy,
                                             scale=acol[:, t:t + 1])
                    else:
                        nc.vector.tensor_scalar_mul(out=ws[t], in0=wtmp[t],
                                                    scalar1=acol[:, t:t + 1])

                # ---------------- bias rows (3 chains: q, k-pair, v) ----------------
                brow_out = {}
                # v row -> vbias
                vrow_ps = ps.tile([1, 512], f32, tag="small", name="vrow_ps", bufs=1)
                for t in range(NT):
                    nc.tensor.matmul(vrow_ps, bsx[:, t:t + 1], wtmp[t][:, 2 * C:3 * C],
                                     start=(t == 0), stop=(t == NT - 1), skip_group_check=True)
                vbrow = rows.tile([1, 512], f32, tag="vbrow")
                nc.scalar.copy(vbrow, vrow_ps)
                vbias = work.tile([128, NT], f32r, tag="vbias")
                for m in range(NT):
                    vtp = ps.tile([128, 1], f32, tag="small", name="vtp", bufs=1,
                                  padded_shape=[128, 512])
                    nc.tensor.transpose(vtp, vbrow[:, 128 * m:128 * (m + 1)], ident[0:1, 0:1])
                    with nc.allow_low_precision(reason="feeds f32r matmul"):
                        nc.vector.tensor_add(vbias[:, m:m + 1], vtp, bvc[:, m:m + 1])
                # q row
                qrow_ps = ps.tile([1, 512], f32, tag="small", name="qrow_ps", bufs=1)
                for t in range(NT):
                    nc.tensor.matmul(qrow_ps, bsx[:, t:t + 1], wtmp[t][:, 0:512],
                                     start=(t == 0), stop=(t == NT - 1), skip_group_check=True)
                browq = rows.tile([1, 512], f32r, tag="browq")
                with nc.allow_low_precision(reason="feeds f32r matmul"):
                    nc.vector.tensor_add(browq, qrow_ps, bqkr[:, 0:512])
                brow_out["q"] = browq
                # k rows (b2 and b2 + a*mean_c together)
                krow_ps = ps.tile([2, 512], f32, tag="small", name="krow_ps", bufs=1)
                for t in range(NT):
                    nc.tensor.matmul(krow_ps, bsx[:, t::4], wtmp[t][:, 512:1024],
                                     start=(t == 0), stop=(t == NT - 1), skip_group_check=True)
                browk = rows.tile([1, 512], f32r, tag="browk")
                with nc.allow_low_precision(reason="feeds f32r matmul"):
                    nc.vector.tensor_add(browk, krow_ps[0:1, :], bqkr[:, 512:1024])
                hkpre = rows.tile([1, 512], f32, tag="hkpre")
                nc.vector.tensor_add(hkpre, krow_ps[1:2, :], bqkr[:, 512:1024])
                hkf = rows.tile([1, 512], f32r, tag="hkf")
                with nc.allow_low_precision(reason="feeds f32r matmul"):
                    nc.scalar.mul(out=hkf, in_=hkpre, mul=float(N))

                # ---------------- lower-triangle blocks of G ----------------
                gT = {}
                for i, (cpb, cb) in enumerate([(1, 0), (2, 0), (3, 0), (2, 1), (3, 1), (3, 2)]):
                    blk = gs[cb][:, 128 * (cpb - cb):128 * (cpb - cb) + 128]
                    gtp = ps.tile([128, 128], f32r, tag="small", name="gtp", bufs=1,
                                  padded_shape=[128, 512])
                    nc.tensor.transpose(gtp, blk, identr)
                    g_t = gpool.tile([128, 128], f32r, tag=f"gt{cpb}{cb}", name=f"gt{cpb}{cb}")
                    if i % 2 == 0:
                        nc.scalar.copy(g_t, gtp)
                    else:
                        nc.vector.tensor_copy(g_t, gtp)
                    gT[(cpb, cb)] = g_t

                def g_stat(cpb, cb):
                    if cpb <= cb:
                        return gs[cpb][:, 128 * (cb - cpb):128 * (cb - cpb) + 128]
                    return gT[(cpb, cb)]

                # ---------------- wsvT: transpose of the v-weight blocks ----------------
                wsvT = []
                for p in range(NT):
                    wtps = ps.tile([128, 512], f32r, tag="tail", name="wtps", bufs=2)
                    for t in range(NT):
                        nc.tensor.transpose(wtps[:, 128 * t:128 * (t + 1)],
                                            ws[t][:, 2 * C + 128 * p:2 * C + 128 * (p + 1)],
                                            identr)
                    wsv_p = gpool.tile([128, 512], f32r, tag=f"wsvT{p}", name=f"wsvT{p}")
                    if p % 2 == 0:
                        nc.scalar.copy(wsv_p, wtps.bitcast(f32))
                    else:
                        nc.vector.tensor_copy(wsv_p, wtps)
                    wsvT.append(wsv_p)

                # ---------------- Tk = G Wk'^T + Sx (x) Bk ----------------
                tks = []
                for cb in range(NT):
                    tk = ps.tile([128, 512], f32, tag="tail", name=f"tk{cb}", bufs=2)
                    for cpb in range(NT):
                        nc.tensor.matmul(tk, g_stat(cpb, cb),
                                         ws[cpb][:, 512:1024], start=(cpb == 0), stop=False)
                    nc.tensor.matmul(tk, sxrow_l[cb], browk, start=False, stop=True)
                    t_s = gpool.tile([128, 512], f32r, tag=f"tks{cb}", name=f"tks{cb}")
                    if cb % 2 == 0:
                        nc.scalar.activation(out=t_s, in_=tk, func=AF.Identity)
                    else:
                        nc.vector.tensor_copy(t_s, tk)
                    tks.append(t_s)

                # ---------------- scores (head pairs, diag blocks used) ----------------
                # 256-wide moving window keeps f32r at 1 cyc/row; pair p's block
                # sits at uoff.
                scps_l = []
                for p in range(NT):
                    roff = min(128 * p, 256)
                    uoff = 128 * p - roff
                    scp = ps.tile([128, 256], f32, tag="tail", name=f"scps{p}", bufs=2,
                                  padded_shape=[128, 512])
                    for cb in range(NT):
                        nc.tensor.matmul(scp, ws[cb][:, 128 * p:128 * (p + 1)],
                                         tks[cb][:, roff:roff + 256],
                                         start=(cb == 0), stop=False, skip_group_check=True)
                    nc.tensor.matmul(scp, browq[:, 128 * p:128 * (p + 1)],
                                     hkf[:, roff:roff + 256], start=False, stop=True,
                                     skip_group_check=True)
                    scps_l.append(scp[:, uoff:uoff + 128])

                # ---------------- softmax (per head pair) -> rden-scaled E ----------------
                e_sl = [work.tile([128, 128], bf16, tag=f"es{p}", name=f"es{p}", bufs=1)
                        for p in range(NT)]
                rden = work.tile([128, NT], f32, tag="rden")
                for p in range(NT):
                    mx = work.tile([128, 1], f32, tag="mx")
                    nc.vector.reduce_max(out=mx[0:64, :], in_=scps_l[p][0:64, 0:64], axis=AX.X)
                    nc.vector.reduce_max(out=mx[64:128, :], in_=scps_l[p][64:128, 64:128], axis=AX.X)
                    negmx = work.tile([128, 1], f32, tag="negmx")
                    nc.scalar.mul(out=negmx, in_=mx, mul=-0.125)
                    e = work.tile([128, 128], f32, tag="exp")
                    nc.vector.memset(e, 0.0)
                    nc.scalar.activation(out=e[0:64, 0:64], in_=scps_l[p][0:64, 0:64],
                                         func=AF.Exp, scale=0.125, bias=negmx[0:64, :])
                    nc.scalar.activation(out=e[64:128, 64:128], in_=scps_l[p][64:128, 64:128],
                                         func=AF.Exp, scale=0.125, bias=negmx[64:128, :])
                    den = work.tile([128, 1], f32, tag="den")
                    nc.vector.reduce_sum(out=den[0:64, :], in_=e[0:64, 0:64], axis=AX.X)
                    nc.vector.reduce_sum(out=den[64:128, :], in_=e[64:128, 64:128], axis=AX.X)
                    nc.vector.reciprocal(rden[:, p:p + 1], den)
                    nc.scalar.activation(out=e_sl[p], in_=e, func=AF.Copy, scale=rden[:, p:p + 1])

                # ---------------- UT[d,o] = sum_c es[c,d] Wp[o,c] (per pair) ----------------
                uts = []
                for p in range(NT):
                    ut_ps = ps.tile([128, 512], f32, tag="tail", name="ut_ps", bufs=2)
                    nc.tensor.matmul(ut_ps, e_sl[p], wp[p], start=True, stop=True)
                    ut_s = gpool.tile([128, 512], bf16, tag=f"uts{p}", name=f"uts{p}")
                    if p % 2 == 0:
                        nc.scalar.activation(out=ut_s, in_=ut_ps, func=AF.Identity)
                    else:
                        nc.vector.tensor_copy(ut_s, ut_ps)
                    uts.append(ut_s)

                # ---------------- MT[c,o] -> M8/Mlo (fp8, DoubleRow packed) ----------------
                # M8 tile [128, 2, 1024]: [kp, i, 512h + o] = 32*MT[kp + 128i + 256h, o]
                M8 = gpool.tile([128, 2, 1024], fp8, tag="M8", name="M8")
                Mlo = gpool.tile([128, 2, 1024], fp8, tag="Mlo", name="Mlo")
                for cb in range(NT):
                    mt_ps = ps.tile([128, 512], f32, tag="tail", name=f"mt_ps{cb}", bufs=2)
                    for p in range(NT):
                        nc.tensor.matmul(mt_ps, wsvT[p][:, 128 * cb:128 * (cb + 1)], uts[p],
                                         start=(p == 0), stop=(p == 3))
                    i, h = cb & 1, cb >> 1
                    with nc.allow_low_precision(reason="fp8 split-GEMM operands"):
                        nc.scalar.activation(out=M8[:, i, 512 * h:512 * (h + 1)], in_=mt_ps,
                                             func=AF.Copy, scale=SS)
                        nc.vector.scalar_tensor_tensor(
                            out=Mlo[:, i, 512 * h:512 * (h + 1)], in0=mt_ps, scalar=SS,
                            in1=M8[:, i, 512 * h:512 * (h + 1)],
                            op0=ALU.mult, op1=ALU.subtract)

                # ---------------- output bias col: bp + UT^T vb ----------------
                ob_ps = ps.tile([1, 512], f32, tag="small", name="ob_ps", bufs=1)
                for p in range(NT):
                    nc.tensor.matmul(ob_ps, vbias[:, p:p + 1], uts[p],
                                     start=(p == 0), stop=(p == 3), skip_group_check=True)
                obrow = rows.tile([1, 512], f32, tag="obrow")
                nc.scalar.copy(obrow, ob_ps)
                tbias = work.tile([128, NT], f32, tag="tbias")
                for m in range(NT):
                    obt = ps.tile([128, 1], f32, tag="small", name="obt", bufs=1,
                                  padded_shape=[128, 512])
                    nc.tensor.transpose(obt, obrow[:, 128 * m:128 * (m + 1)], ident[0:1, 0:1])
                    nc.vector.tensor_add(tbias[:, m:m + 1], obt, bpc_t[:, m:m + 1])

                # ---------------- fp8 split GEMM: 3 DoubleRow chains + bias ----------------
                for nj in range(NJ):
                    qj, oj = nj // 2, 1024 * (nj % 2)
                    for m in range(NT):
                        pps = ps.tile([128, 512], f32, tag="pps", name="pps", bufs=2)
                        first = True
                        for lhs, rhs in ((M8, x8t), (M8, e8t), (Mlo, x8t)):
                            for h in range(2):
                                nc.tensor.matmul(
                                    pps,
                                    lhs[:, :, 512 * h + 128 * m:512 * h + 128 * (m + 1)],
                                    rhs[h][qj][:, :, oj // 2:oj // 2 + 512],
                                    start=first, stop=False,
                                    perf_mode=DR, skip_group_check=True)
                                first = False
                        # residual: exact 32*I chains against x8+e8
                        iv = m & 1
                        nc.tensor.matmul(pps, I32[:, iv:iv + 2, :],
                                         x8t[m >> 1][qj][:, :, oj // 2:oj // 2 + 512],
                                         start=False, stop=False,
                                         perf_mode=DR, skip_group_check=True)
                        nc.tensor.matmul(pps, I32[:, iv:iv + 2, :],
                                         e8t[m >> 1][qj][:, :, oj // 2:oj // 2 + 512],
                                         start=False, stop=True,
                                         perf_mode=DR, skip_group_check=True)
                        stage = stagepool.tile([128, 512], bf16, tag="stage", bufs=16)
                        with nc.allow_low_precision(reason="bf16 output store"):
                            if (nj + m) % 2 == 0:
                                nc.scalar.activation(out=stage, in_=pps, func=AF.Identity,
                                                     scale=1.0 / SS, bias=tbias[:, m:m + 1])
                            else:
                                nc.vector.tensor_scalar(out=stage, in0=pps,
                                                        scalar1=1.0 / SS,
                                                        scalar2=tbias[:, m:m + 1],
                                                        op0=ALU.mult, op1=ALU.add)
                        out_eng = nc.sync if (nj + m) % 2 == 0 else nc.gpsimd
                        out_eng.dma_start(
                            out=out2[b, 128 * m:128 * (m + 1), 512 * nj:512 * (nj + 1)],
                            in_=stage)

    nc.compile()
    return nc


def _get_nc():
    if "nc" not in _cache:
        _cache["nc"] = _build()
    return _cache["nc"]


def _prep_core_inputs(x_core, gamma, beta, w_qkv, b_qkv, w_proj, b_proj):
    """Host-side input prep for one core. x_core: [BPC, C, H, W] or [BPC, C, N] f32."""
    import ml_dtypes
    f8 = ml_dtypes.float8_e4m3
    xr = np.ascontiguousarray(np.asarray(x_core, np.float32).reshape(BPC, C, N))
    xbf = xr.astype(ml_dtypes.bfloat16)
    xbf32 = xbf.astype(np.float32)
    x8 = xbf32.astype(f8)
    e8 = (xbf32 - x8.astype(np.float32)).astype(f8)
    gmask_d = np.zeros((128, 8), dtype=np.float32)
    gmask_d[np.arange(128), np.arange(128) // 16] = 1.0
    return {
        "x2bf": xbf, "x8d": x8, "e8d": e8,
        "wqkT": np.ascontiguousarray(np.asarray(w_qkv, np.float32).T),
        "wpT": np.ascontiguousarray(np.asarray(w_proj, np.float32).T),
        "gamma_pc": np.ascontiguousarray(np.asarray(gamma, np.float32).reshape(NT, 128).T),
        "beta_pc": np.ascontiguousarray(np.asarray(beta, np.float32).reshape(NT, 128).T),
        "bqk_row": np.ascontiguousarray(np.asarray(b_qkv, np.float32)[:2 * C].reshape(1, 2 * C)),
        "bv_pc": np.ascontiguousarray(np.asarray(b_qkv, np.float32)[2 * C:].reshape(NT, 128).T),
        "bp_pc": np.ascontiguousarray(np.asarray(b_proj, np.float32).reshape(NT, 128).T),
        "ident_d": np.eye(128, dtype=np.float32),
        "gmask_d": gmask_d,
        "gmaskT_d": np.ascontiguousarray(gmask_d.T),
    }


def kernel(x, gamma, beta, w_qkv, b_qkv, w_proj, b_proj):
    from concourse.bass_utils import run_bass_kernel_spmd

    x = np.asarray(x, dtype=np.float32)
    nc = _get_nc()

    in_maps = []
    for i in range(NCORES):
        in_maps.append(_prep_core_inputs(
            x[BPC * i:BPC * (i + 1)], gamma, beta, w_qkv, b_qkv, w_proj, b_proj))

    res = run_bass_kernel_spmd(nc, in_maps, core_ids=list(range(NCORES)))
    out = np.empty((B, C, N), dtype=np.float32)
    for i in range(NCORES):
        out[BPC * i:BPC * (i + 1)] = np.asarray(res.results[i]["out2"], dtype=np.float32)
    return out.reshape(B, C, H, W)


# revision 23
# speedup vs baseline: 1.2187x; 1.2187x over previous
"""Trainium2 Bass kernel for nn_AttentionBlock (GroupNorm + qkv conv + head-dim attention + proj + residual).

Sharding: data-parallel over batch B=16 -> 2 batch elements per core on 8 cores.

Structure (per batch element). The attention contracts over PIXELS (scores are
[64,64] per head), so q,k,v are never materialized per-pixel:
  G    = X X^T            bf16 Gram from DMA-transposed x chunks (no PE
                          transposes, no engine transpose copies)
  stats: channel sums ride the Gram as 4 extra ones-columns; channel sum(x^2)
         comes off the Gram diagonal (diag-block * I, row-reduce).  GroupNorm
         mean/rstd via the gmask matmuls.  No bn_stats pass over x.
  Tk   = G Wk'^T + Sx (x) Bk    (f32r, exact in sim)
  S_p  = Wq'^T Tk + Bq (x) hk   per-head-pair scores (f32r)
  E    = softmax(S/8)           rden folded into E (bf16)
  UT   = E'^T Wp^T ; MT = Wv'^T UT  -> M8 = fp8(32*MT), Mlo = fp8(32*MT - M8)
  out  = [M8^T(x8+e8) + Mlo^T x8]/32 + tbias + residual
         3 fp8 DoubleRow chains (2 steps each) instead of 4 bf16 steps.
         x8 = fp8(x), e8 = fp8(x - x8) are host-prepared; residual lands in
         out2 via an early DRAM->DRAM cast copy, and the projection output is
         DMA-accumulated on top (gpsimd SWDGE).
GroupNorm is folded into the weights (Wq' = Wq diag(a), biases via b2 = beta -
mean*a); x is never normalized in memory.
"""
import sys, os
sys.path.insert(0, "/opt/trn_rl_repo")
sys.path.insert(0, "/opt/trn_rl_repo/concourse")
import numpy as np

B, C, H, W = 16, 512, 64, 64
N = H * W            # 4096 spatial
NH = 8               # heads
D = C // NH          # 64 head dim
G = 32               # groups
EPS = 1e-5
NCORES = 8
BPC = B // NCORES    # 2 batches per core

NT = C // 128        # 4 channel tiles
NCHUNK = N // 128    # 32 pixel chunks
NJ = N // 512        # 8 column blocks of 512
SS = 32.0            # fp8 M scale

_cache = {}


def _build():
    import concourse.bass as bass
    import concourse.bacc as bacc
    import concourse.tile as tile
    from concourse import mybir
    from concourse.masks import make_identity

    f32 = mybir.dt.float32
    f32r = mybir.dt.float32r
    bf16 = mybir.dt.bfloat16
    fp8 = mybir.dt.float8e4
    AF = mybir.ActivationFunctionType
    ALU = mybir.AluOpType
    AX = mybir.AxisListType
    DR = mybir.MatmulPerfMode.DoubleRow

    nc = bacc.Bacc()

    x2bf = nc.dram_tensor("x2bf", [BPC, C, N], bf16, kind="ExternalInput")
    x8d = nc.dram_tensor("x8d", [BPC, C, N], fp8, kind="ExternalInput")
    e8d = nc.dram_tensor("e8d", [BPC, C, N], fp8, kind="ExternalInput")
    wqkT = nc.dram_tensor("wqkT", [C, 3 * C], f32r, kind="ExternalInput")  # w_qkv.T  [c, o]
    wpT = nc.dram_tensor("wpT", [C, C], f32, kind="ExternalInput")          # w_proj.T [c, o]
    gamma_pc = nc.dram_tensor("gamma_pc", [128, NT], f32, kind="ExternalInput")
    beta_pc = nc.dram_tensor("beta_pc", [128, NT], f32, kind="ExternalInput")
    bqk_row = nc.dram_tensor("bqk_row", [1, 2 * C], f32, kind="ExternalInput")
    bv_pc = nc.dram_tensor("bv_pc", [128, NT], f32, kind="ExternalInput")
    bp_pc = nc.dram_tensor("bp_pc", [128, NT], f32, kind="ExternalInput")
    ident_d = nc.dram_tensor("ident_d", [128, 128], f32r, kind="ExternalInput")
    gmask_d = nc.dram_tensor("gmask_d", [128, 8], f32r, kind="ExternalInput")
    gmaskT_d = nc.dram_tensor("gmaskT_d", [8, 128], f32r, kind="ExternalInput")
    out2 = nc.dram_tensor("out2", [BPC, C, N], bf16, kind="ExternalOutput")

    GXW = [512, 384, 256, 128]   # true upper-triangle widths per row block

    with tile.TileContext(nc) as tc:
        with tc.tile_pool(name="consts", bufs=1) as consts, \
             tc.tile_pool(name="wpool", bufs=1) as wpool, \
             tc.tile_pool(name="xpool", bufs=1) as xpool, \
             tc.tile_pool(name="gpool", bufs=1) as gpool, \
             tc.tile_pool(name="xtcpool", bufs=1) as xtcpool, \
             tc.tile_pool(name="rows", bufs=1) as rows, \
             tc.tile_pool(name="work", bufs=2) as work, \
             tc.tile_pool(name="stagepool", bufs=2) as stagepool, \
             tc.tile_pool(name="ps", bufs=1, space="PSUM") as ps:

            # ---------------- constants / weights (once per core) ----------------
            ident = consts.tile([128, 128], f32, tag="ident")
            make_identity(nc, ident)
            identr = consts.tile([128, 128], f32r, tag="identr")
            nc.gpsimd.dma_start(out=identr, in_=ident_d[:, :])
            gmask = consts.tile([128, 8], f32r, tag="gmask")
            nc.gpsimd.dma_start(out=gmask, in_=gmask_d[:, :])
            gmaskT = consts.tile([8, 128], f32r, tag="gmaskT")
            nc.gpsimd.dma_start(out=gmaskT, in_=gmaskT_d[:, :])
            onescol = consts.tile([128, 1], bf16, tag="onescol")
            nc.vector.memset(onescol, 1.0)
            epst8 = consts.tile([8, 1], f32, tag="epst8")
            nc.vector.memset(epst8, EPS)
            # residual identity for the fp8 DoubleRow GEMM: [:, 0:2, :] selects
            # (32*I, 0) for even m blocks, [:, 1:3, :] selects (0, 32*I) for odd.
            I32 = consts.tile([128, 3, 128], fp8, tag="I32")
            nc.vector.memset(I32, 0.0)
            with nc.allow_low_precision(reason="fp8 exact powers of two"):
                nc.scalar.activation(out=I32[:, 0, :], in_=ident, func=AF.Copy, scale=SS)
                nc.scalar.activation(out=I32[:, 2, :], in_=ident, func=AF.Copy, scale=SS)

            bpc_t = consts.tile([128, NT], f32, tag="bpc_t")
            nc.gpsimd.dma_start(out=bpc_t, in_=bp_pc[:, :])
            gam = consts.tile([128, NT], f32, tag="gam")
            nc.gpsimd.dma_start(out=gam, in_=gamma_pc[:, :])
            bet = consts.tile([128, NT], f32, tag="bet")
            nc.gpsimd.dma_start(out=bet, in_=beta_pc[:, :])
            bvc = consts.tile([128, NT], f32, tag="bvc")
            nc.gpsimd.dma_start(out=bvc, in_=bv_pc[:, :])
            bqkr = consts.tile([1, 2 * C], f32, tag="bqkr")
            nc.gpsimd.dma_start(out=bqkr, in_=bqk_row[:, :])

            wtmp = []
            ws = []
            wp = []
            for t in range(NT):
                w_t = wpool.tile([128, 3 * C], f32r, tag=f"wtmp{t}", name=f"wtmp{t}")
                nc.gpsimd.dma_start(out=w_t, in_=wqkT[128 * t:128 * (t + 1), :])
                wtmp.append(w_t)
                ws_t = wpool.tile([128, 3 * C], f32r, tag=f"ws{t}", name=f"ws{t}")
                ws.append(ws_t)
                w_p = wpool.tile([128, C], bf16, tag=f"wp{t}", name=f"wp{t}")
                nc.gpsimd.dma_start(out=w_p, in_=wpT[128 * t:128 * (t + 1), :])
                wp.append(w_p)

            for b in range(BPC):
                # ---------------- DMA: fused transposed x + fp8 x loads ----------------
                # 4 big DMA transposes per batch: [512, 1024] -> [128, 8, 512]
                # (xg[:, j, :] = chunk (8*g + j) in [px, ch] layout)
                xtcg = []
                for g in range(4):
                    xg = xtcpool.tile([128, 8, C], bf16, tag=f"xtcg{g}", name=f"xtcg{g}")
                    nc.sync.dma_start(out=xg,
                                      in_=x2bf[b, :, 1024 * g:1024 * (g + 1)],
                                      transpose=True)
                    xtcg.append(xg)
                xtc_l = [xtcg[ni // 8][:, ni % 8, :] for ni in range(NCHUNK)]

                x8t = [None, None]
                e8t = [None, None]
                for h in range(2):
                    x_t = xpool.tile([128, 2, N], fp8, tag=f"x8{h}", name=f"x8{h}", bufs=1)
                    nc.sync.dma_start(
                        out=x_t,
                        in_=x8d[b, 256 * h:256 * (h + 1), :]
                        .rearrange("(i k) n -> k i n", i=2))
                    x8t[h] = x_t
                    e_t = xpool.tile([128, 2, N], fp8, tag=f"e8{h}", name=f"e8{h}", bufs=1)
                    nc.scalar.dma_start(
                        out=e_t,
                        in_=e8d[b, 256 * h:256 * (h + 1), :]
                        .rearrange("(i k) n -> k i n", i=2))
                    e8t[h] = e_t

                # ---------------- Gram (bf16) + channel-sum columns ----------------
                # gxA: rows 0:128  cols 0:512   (bank 1)
                # gxB: rows 128:256 cols 128:512 (bank 2)
                # gxCD: rows 256:384 cols 256:512 at [:,0:256];
                #       rows 384:512 cols 384:512 at [:,256:384];
                #       channel sums at [:,384:388]          (bank 3)
                gxA = ps.tile([128, 512], f32, tag="gxA", name="gxA", bufs=1)
                gxB = ps.tile([128, 512], f32, tag="gxB", name="gxB", bufs=1)
                gxCD = ps.tile([128, 512], f32, tag="gxCD", name="gxCD", bufs=1)

                for ni in range(NCHUNK):
                    xtc = xtc_l[ni]
                    st = (ni == 0)
                    sp = (ni == NCHUNK - 1)
                    nc.tensor.matmul(gxA, xtc[:, 0:128], xtc[:, 0:512],
                                     start=st, stop=sp, skip_group_check=True)
                    nc.tensor.matmul(gxB[:, 0:384], xtc[:, 128:256], xtc[:, 128:512],
                                     start=st, stop=sp, skip_group_check=True)
                    nc.tensor.matmul(gxCD[:, 0:256], xtc[:, 256:384], xtc[:, 256:512],
                                     start=st, stop=False, skip_group_check=True)
                    nc.tensor.matmul(gxCD[:, 256:384], xtc[:, 384:512], xtc[:, 384:512],
                                     start=False, stop=False, skip_group_check=True)
                    for cb in range(NT):
                        nc.tensor.matmul(gxCD[:, 384 + cb:385 + cb],
                                         xtc[:, 128 * cb:128 * (cb + 1)], onescol,
                                         start=False, stop=sp and (cb == NT - 1),
                                         skip_group_check=True)

                # ---------------- drain G to SBUF (f32r), sums to S8 ----------------
                gx_src = [gxA[:, 0:512], gxB[:, 0:384], gxCD[:, 0:256], gxCD[:, 256:384]]
                gs = []
                for cb in range(NT):
                    g_s = gpool.tile([128, GXW[cb]], f32r, tag=f"gs{cb}", name=f"gs{cb}")
                    if cb % 2 == 0:
                        nc.scalar.activation(out=g_s, in_=gx_src[cb], func=AF.Identity)
                    else:
                        nc.vector.tensor_copy(g_s, gx_src[cb])
                    gs.append(g_s)
                S8 = work.tile([128, 8], f32r, tag="S8", bufs=1)
                with nc.allow_low_precision(reason="sums feed f32r matmuls"):
                    nc.scalar.activation(out=S8[:, 0:4], in_=gxCD[:, 384:388], func=AF.Identity)
                    # diag(G) per row block: mask with identity, row-reduce
                    for cb in range(NT):
                        dsq = work.tile([128, 128], f32r, tag="dsq", name="dsq", bufs=2)
                        nc.vector.tensor_tensor(dsq, gs[cb][:, 0:128], ident, op=ALU.mult)
                        nc.vector.reduce_sum(out=S8[:, 4 + cb:5 + cb], in_=dsq, axis=AX.X)

                # ---------------- group stats via mask matmuls ----------------
                gsum_ps = ps.tile([8, 8], f32, tag="small", name="gsum_ps", bufs=1,
                                  padded_shape=[8, 512])
                nc.tensor.matmul(gsum_ps, gmask, S8, start=True, stop=True,
                                 skip_group_check=True)
                mg8 = work.tile([8, 8], f32r, tag="mg8")
                with nc.allow_low_precision(reason="feeds f32r matmul"):
                    nc.scalar.mul(out=mg8[:, 0:4], in_=gsum_ps[:, 0:4], mul=1.0 / (16.0 * N))
                ex2 = work.tile([8, 4], f32, tag="ex2")
                nc.scalar.mul(out=ex2, in_=gsum_ps[:, 4:8], mul=1.0 / (16.0 * N))
                msq = work.tile([8, 4], f32, tag="msq")
                nc.vector.tensor_tensor(msq, mg8[:, 0:4].bitcast(f32), mg8[:, 0:4].bitcast(f32),
                                        op=ALU.mult)
                var_g = work.tile([8, 4], f32, tag="var_g")
                nc.vector.tensor_tensor(var_g, ex2, msq, op=ALU.subtract)
                sdg = work.tile([8, 4], f32, tag="sdg")
                nc.scalar.activation(out=sdg, in_=var_g, func=AF.Sqrt, bias=epst8)
                with nc.allow_low_precision(reason="feeds f32r matmul"):
                    nc.vector.reciprocal(mg8[:, 4:8], sdg)
                pcmr = ps.tile([128, 8], f32, tag="small", name="pcmr", bufs=1,
                               padded_shape=[128, 512])
                nc.tensor.matmul(pcmr, gmaskT, mg8, start=True, stop=True,
                                 skip_group_check=True)
                acol = work.tile([128, NT], f32, tag="acol")
                nc.vector.tensor_tensor(acol, pcmr[:, 4:8], gam, op=ALU.mult)
                # bsx cols 0:4 = b2 = beta - mean_g*a ; cols 4:8 = b2 + a*mean_c
                bsx = rows.tile([128, 8], f32r, tag="bsx")
                tmpb = work.tile([128, NT], f32, tag="tmpb")
                nc.vector.tensor_tensor(tmpb, pcmr[:, 0:4], acol, op=ALU.mult)
                with nc.allow_low_precision(reason="feeds f32r matmul"):
                    nc.vector.tensor_tensor(bsx[:, 0:4], bet, tmpb, op=ALU.subtract)
                amv = work.tile([128, NT], f32, tag="amv")
                nc.vector.tensor_tensor(amv, acol, S8[:, 0:4].bitcast(f32), op=ALU.mult)
                with nc.allow_low_precision(reason="feeds f32r matmul"):
                    nc.vector.scalar_tensor_tensor(
                        out=bsx[:, 4:8], in0=amv, scalar=1.0 / N,
                        in1=bsx[:, 0:4].bitcast(f32), op0=ALU.mult, op1=ALU.add)
                # channel-sum rows for the rank-1 score terms
                sxrow_l = []
                for t in range(NT):
                    sxtp = ps.tile([1, 128], f32, tag="small", name="sxtp", bufs=1,
                                   padded_shape=[1, 512])
                    nc.tensor.transpose(sxtp, S8[:, t:t + 1].bitcast(f32), ident)
                    sxrow = rows.tile([1, 128], f32r, tag=f"sxrow{t}", name=f"sxrow{t}")
                    with nc.allow_low_precision(reason="feeds f32r matmul"):
                        nc.scalar.mul(out=sxrow, in_=sxtp, mul=1.0)
                    sxrow_l.append(sxrow)

                # ---------------- ws = wtmp * acol (split ACT/DVE) ----------------
                for t in range(NT):
                    if t % 2 == 0:
                        nc.scalar.activation(out=ws[t], in_=wtmp[t], func=AF.Copy,
                                             scale=acol[:, t:t + 1])
                    else:
                        nc.vector.tensor_scalar_mul(out=ws[t], in0=wtmp[t],
                                                    scalar1=acol[:, t:t + 1])

                # ---------------- bias rows (3 chains: q, k-pair, v) ----------------
                brow_out = {}
                # v row -> vbias
                vrow_ps = ps.tile([1, 512], f32, tag="small", name="vrow_ps", bufs=1)
                for t in range(NT):
                    nc.tensor.matmul(vrow_ps, bsx[:, t:t + 1], wtmp[t][:, 2 * C:3 * C],
                                     start=(t == 0), stop=(t == NT - 1), skip_group_check=True)
                vbrow = rows.tile([1, 512], f32, tag="vbrow")
                nc.scalar.copy(vbrow, vrow_ps)
                vbias = work.tile([128, NT], f32r, tag="vbias")
                for m in range(NT):
                    vtp = ps.tile([128, 1], f32, tag="small", name="vtp", bufs=1,
                                  padded_shape=[128, 512])
                    nc.tensor.transpose(vtp, vbrow[:, 128 * m:128 * (m + 1)], ident[0:1, 0:1])
                    with nc.allow_low_precision(reason="feeds f32r matmul"):
                        nc.vector.tensor_add(vbias[:, m:m + 1], vtp, bvc[:, m:m + 1])
                # q row
                qrow_ps = ps.tile([1, 512], f32, tag="small", name="qrow_ps", bufs=1)
                for t in range(NT):
                    nc.tensor.matmul(qrow_ps, bsx[:, t:t + 1], wtmp[t][:, 0:512],
                                     start=(t == 0), stop=(t == NT - 1), skip_group_check=True)
                browq = rows.tile([1, 512], f32r, tag="browq")
                with nc.allow_low_precision(reason="feeds f32r matmul"):
                    nc.vector.tensor_add(browq, qrow_ps, bqkr[:, 0:512])
                brow_out["q"] = browq
                # k rows (b2 and b2 + a*mean_c together)
                krow_ps = ps.tile([2, 512], f32, tag="small", name="krow_ps", bufs=1)
                for t in range(NT):
                    nc.tensor.matmul(krow_ps, bsx[:, t::4], wtmp[t][:, 512:1024],
                                     start=(t == 0), stop=(t == NT - 1), skip_group_check=True)
                browk = rows.tile([1, 512], f32r, tag="browk")
                with nc.allow_low_precision(reason="feeds f32r matmul"):
                    nc.vector.tensor_add(browk, krow_ps[0:1, :], bqkr[:, 512:1024])
                hkpre = rows.tile([1, 512], f32, tag="hkpre")
                nc.vector.tensor_add(hkpre, krow_ps[1:2, :], bqkr[:, 512:1024])
                hkf = rows.tile([1, 512], f32r, tag="hkf")
                with nc.allow_low_precision(reason="feeds f32r matmul"):
                    nc.scalar.mul(out=hkf, in_=hkpre, mul=float(N))

                # ---------------- lower-triangle blocks of G ----------------
                gT = {}
                for i, (cpb, cb) in enumerate([(1, 0), (2, 0), (3, 0), (2, 1), (3, 1), (3, 2)]):
                    blk = gs[cb][:, 128 * (cpb - cb):128 * (cpb - cb) + 128]
                    gtp = ps.tile([128, 128], f32r, tag="small", name="gtp", bufs=1,
                                  padded_shape=[128, 512])
                    nc.tensor.transpose(gtp, blk, identr)
                    g_t = gpool.tile([128, 128], f32r, tag=f"gt{cpb}{cb}", name=f"gt{cpb}{cb}")
                    if i % 2 == 0:
                        nc.scalar.copy(g_t, gtp)
                    else:
                        nc.vector.tensor_copy(g_t, gtp)
                    gT[(cpb, cb)] = g_t

                def g_stat(cpb, cb):
                    if cpb <= cb:
                        return gs[cpb][:, 128 * (cb - cpb):128 * (cb - cpb) + 128]
                    return gT[(cpb, cb)]

                # ---------------- wsvT: transpose of the v-weight blocks ----------------
                wsvT = []
                for p in range(NT):
                    wtps = ps.tile([128, 512], f32r, tag="tail", name="wtps", bufs=2)
                    for t in range(NT):
                        nc.tensor.transpose(wtps[:, 128 * t:128 * (t + 1)],
                                            ws[t][:, 2 * C + 128 * p:2 * C + 128 * (p + 1)],
                                            identr)
                    wsv_p = gpool.tile([128, 512], f32r, tag=f"wsvT{p}", name=f"wsvT{p}")
                    if p % 2 == 0:
                        nc.scalar.copy(wsv_p, wtps.bitcast(f32))
                    else:
                        nc.vector.tensor_copy(wsv_p, wtps)
                    wsvT.append(wsv_p)

                # ---------------- Tk = G Wk'^T + Sx (x) Bk ----------------
                tks = []
                for cb in range(NT):
                    tk = ps.tile([128, 512], f32, tag="tail", name=f"tk{cb}", bufs=2)
                    for cpb in range(NT):
                        nc.tensor.matmul(tk, g_stat(cpb, cb),
                                         ws[cpb][:, 512:1024], start=(cpb == 0), stop=False)
                    nc.tensor.matmul(tk, sxrow_l[cb], browk, start=False, stop=True)
                    t_s = gpool.tile([128, 512], f32r, tag=f"tks{cb}", name=f"tks{cb}")
                    if cb % 2 == 0:
                        nc.scalar.activation(out=t_s, in_=tk, func=AF.Identity)
                    else:
                        nc.vector.tensor_copy(t_s, tk)
                    tks.append(t_s)

                # ---------------- scores (head pairs, diag blocks used) ----------------
                # 256-wide moving window keeps f32r at 1 cyc/row; pair p's block
                # sits at uoff.
                scps_l = []
                for p in range(NT):
                    roff = min(128 * p, 256)
                    uoff = 128 * p - roff
                    scp = ps.tile([128, 256], f32, tag="tail", name=f"scps{p}", bufs=2,
                                  padded_shape=[128, 512])
                    for cb in range(NT):
                        nc.tensor.matmul(scp, ws[cb][:, 128 * p:128 * (p + 1)],
                                         tks[cb][:, roff:roff + 256],
                                         start=(cb == 0), stop=False, skip_group_check=True)
                    nc.tensor.matmul(scp, browq[:, 128 * p:128 * (p + 1)],
                                     hkf[:, roff:roff + 256], start=False, stop=True,
                                     skip_group_check=True)
                    scps_l.append(scp[:, uoff:uoff + 128])

                # ---------------- softmax (per head pair) -> rden-scaled E ----------------
                e_sl = [work.tile([128, 128], bf16, tag=f"es{p}", name=f"es{p}", bufs=1)
                        for p in range(NT)]
                rden = work.tile([128, NT], f32, tag="rden")
                for p in range(NT):
                    mx = work.tile([128, 1], f32, tag="mx")
                    nc.vector.reduce_max(out=mx[0:64, :], in_=scps_l[p][0:64, 0:64], axis=AX.X)
                    nc.vector.reduce_max(out=mx[64:128, :], in_=scps_l[p][64:128, 64:128], axis=AX.X)
                    negmx = work.tile([128, 1], f32, tag="negmx")
                    nc.scalar.mul(out=negmx, in_=mx, mul=-0.125)
                    e = work.tile([128, 128], f32, tag="exp")
                    nc.vector.memset(e, 0.0)
                    nc.scalar.activation(out=e[0:64, 0:64], in_=scps_l[p][0:64, 0:64],
                                         func=AF.Exp, scale=0.125, bias=negmx[0:64, :])
                    nc.scalar.activation(out=e[64:128, 64:128], in_=scps_l[p][64:128, 64:128],
                                         func=AF.Exp, scale=0.125, bias=negmx[64:128, :])
                    den = work.tile([128, 1], f32, tag="den")
                    nc.vector.reduce_sum(out=den[0:64, :], in_=e[0:64, 0:64], axis=AX.X)
                    nc.vector.reduce_sum(out=den[64:128, :], in_=e[64:128, 64:128], axis=AX.X)
                    nc.vector.reciprocal(rden[:, p:p + 1], den)
                    nc.scalar.activation(out=e_sl[p], in_=e, func=AF.Copy, scale=rden[:, p:p + 1])

                # ---------------- UT[d,o] = sum_c es[c,d] Wp[o,c] (per pair) ----------------
                uts = []
                for p in range(NT):
                    ut_ps = ps.tile([128, 512], f32, tag="tail", name="ut_ps", bufs=2)
                    nc.tensor.matmul(ut_ps, e_sl[p], wp[p], start=True, stop=True)
                    ut_s = gpool.tile([128, 512], bf16, tag=f"uts{p}", name=f"uts{p}")
                    if p % 2 == 0:
                        nc.scalar.activation(out=ut_s, in_=ut_ps, func=AF.Identity)
                    else:
                        nc.vector.tensor_copy(ut_s, ut_ps)
                    uts.append(ut_s)

                # ---------------- MT[c,o] -> M8/Mlo (fp8, DoubleRow packed) ----------------
                # M8 tile [128, 2, 1024]: [kp, i, 512h + o] = 32*MT[kp + 128i + 256h, o]
                M8 = gpool.tile([128, 2, 1024], fp8, tag="M8", name="M8")
                Mlo = gpool.tile([128, 2, 1024], fp8, tag="Mlo", name="Mlo")
                for cb in range(NT):
                    mt_ps = ps.tile([128, 512], f32, tag="tail", name=f"mt_ps{cb}", bufs=2)
                    for p in range(NT):
                        nc.tensor.matmul(mt_ps, wsvT[p][:, 128 * cb:128 * (cb + 1)], uts[p],
                                         start=(p == 0), stop=(p == 3))
                    i, h = cb & 1, cb >> 1
                    with nc.allow_low_precision(reason="fp8 split-GEMM operands"):
                        nc.scalar.activation(out=M8[:, i, 512 * h:512 * (h + 1)], in_=mt_ps,
                                             func=AF.Copy, scale=SS)
                        nc.vector.scalar_tensor_tensor(
                            out=Mlo[:, i, 512 * h:512 * (h + 1)], in0=mt_ps, scalar=SS,
                            in1=M8[:, i, 512 * h:512 * (h + 1)],
                            op0=ALU.mult, op1=ALU.subtract)

                # ---------------- output bias col: bp + UT^T vb ----------------
                ob_ps = ps.tile([1, 512], f32, tag="small", name="ob_ps", bufs=1)
                for p in range(NT):
                    nc.tensor.matmul(ob_ps, vbias[:, p:p + 1], uts[p],
                                     start=(p == 0), stop=(p == 3), skip_group_check=True)
                obrow = rows.tile([1, 512], f32, tag="obrow")
                nc.scalar.copy(obrow, ob_ps)
                tbias = work.tile([128, NT], f32, tag="tbias")
                for m in range(NT):
                    obt = ps.tile([128, 1], f32, tag="small", name="obt", bufs=1,
                                  padded_shape=[128, 512])
                    nc.tensor.transpose(obt, obrow[:, 128 * m:128 * (m + 1)], ident[0:1, 0:1])
                    nc.vector.tensor_add(tbias[:, m:m + 1], obt, bpc_t[:, m:m + 1])

                # ---------------- fp8 split GEMM: 3 DoubleRow chains + bias ----------------
                for m in range(NT):
                    for nj in range(NJ):
                        oj = 512 * nj
                        pps = ps.tile([128, 512], f32, tag="pps", name="pps", bufs=2)
                        first = True
                        for lhs, rhs in ((M8, x8t), (M8, e8t), (Mlo, x8t)):
                            for h in range(2):
                                nc.tensor.matmul(
                                    pps,
                                    lhs[:, :, 512 * h + 128 * m:512 * h + 128 * (m + 1)],
                                    rhs[h][:, :, oj:oj + 512],
                                    start=first, stop=False,
                                    perf_mode=DR, skip_group_check=True)
                                first = False
                        # residual: exact 32*I chains against x8+e8
                        iv = m & 1
                        nc.tensor.matmul(pps, I32[:, iv:iv + 2, :],
                                         x8t[m >> 1][:, :, oj:oj + 512],
                                         start=False, stop=False,
                                         perf_mode=DR, skip_group_check=True)
                        nc.tensor.matmul(pps, I32[:, iv:iv + 2, :],
                                         e8t[m >> 1][:, :, oj:oj + 512],
                                         start=False, stop=True,
                                         perf_mode=DR, skip_group_check=True)
                        if nj % 2 == 0:
                            stage = stagepool.tile([128, 1024], bf16, tag="stage", bufs=4)
                        swin = stage[:, 512 * (nj % 2):512 * (nj % 2) + 512]
                        with nc.allow_low_precision(reason="bf16 output store"):
                            if nj % 2 == 0:
                                nc.scalar.activation(out=swin, in_=pps, func=AF.Identity,
                                                     scale=1.0 / SS, bias=tbias[:, m:m + 1])
                            else:
                                nc.vector.tensor_scalar(out=swin, in0=pps,
                                                        scalar1=1.0 / SS,
                                                        scalar2=tbias[:, m:m + 1],
                                                        op0=ALU.mult, op1=ALU.add)
                        if nj % 2 == 1:
                            out_eng = nc.sync if (m + nj // 2) % 2 == 0 else nc.gpsimd
                            out_eng.dma_start(
                                out=out2[b, 128 * m:128 * (m + 1), 512 * (nj - 1):512 * (nj + 1)],
                                in_=stage)

    nc.compile()
    return nc


def _get_nc():
    if "nc" not in _cache:
        _cache["nc"] = _build()
    return _cache["nc"]


def _prep_core_inputs(x_core, gamma, beta, w_qkv, b_qkv, w_proj, b_proj):
    """Host-side input prep for one core. x_core: [BPC, C, H, W] or [BPC, C, N] f32."""
    import ml_dtypes
    f8 = ml_dtypes.float8_e4m3
    xr = np.ascontiguousarray(np.asarray(x_core, np.float32).reshape(BPC, C, N))
    xbf = xr.astype(ml_dtypes.bfloat16)
    xbf32 = xbf.astype(np.float32)
    x8 = xbf32.astype(f8)
    e8 = (xbf32 - x8.astype(np.float32)).astype(f8)
    gmask_d = np.zeros((128, 8), dtype=np.float32)
    gmask_d[np.arange(128), np.arange(128) // 16] = 1.0
    return {
        "x2bf": xbf, "x8d": x8, "e8d": e8,
        "wqkT": np.ascontiguousarray(np.asarray(w_qkv, np.float32).T),
        "wpT": np.ascontiguousarray(np.asarray(w_proj, np.float32).T),
        "gamma_pc": np.ascontiguousarray(np.asarray(gamma, np.float32).reshape(NT, 128).T),
        "beta_pc": np.ascontiguousarray(np.asarray(beta, np.float32).reshape(NT, 128).T),
        "bqk_row": np.ascontiguousarray(np.asarray(b_qkv, np.float32)[:2 * C].reshape(1, 2 * C)),
        "bv_pc": np.ascontiguousarray(np.asarray(b_qkv, np.float32)[2 * C:].reshape(NT, 128).T),
        "bp_pc": np.ascontiguousarray(np.asarray(b_proj, np.float32).reshape(NT, 128).T),
        "ident_d": np.eye(128, dtype=np.float32),
        "gmask_d": gmask_d,
        "gmaskT_d": np.ascontiguousarray(gmask_d.T),
    }


def kernel(x, gamma, beta, w_qkv, b_qkv, w_proj, b_proj):
    from concourse.bass_utils import run_bass_kernel_spmd

    x = np.asarray(x, dtype=np.float32)
    nc = _get_nc()

    in_maps = []
    for i in range(NCORES):
        in_maps.append(_prep_core_inputs(
            x[BPC * i:BPC * (i + 1)], gamma, beta, w_qkv, b_qkv, w_proj, b_proj))

    res = run_bass_kernel_spmd(nc, in_maps, core_ids=list(range(NCORES)))
    out = np.empty((B, C, N), dtype=np.float32)
    for i in range(NCORES):
        out[BPC * i:BPC * (i + 1)] = np.asarray(res.results[i]["out2"], dtype=np.float32)
    return out.reshape(B, C, H, W)
